# revision 35
# baseline (speedup 1.0000x reference)
"""Trainium2 Bass kernel for DifferentiableDLT (batched weighted-DLT homography fit).

Contract: kernel(**inputs) takes FULL inputs
    flow (64, 2, 320, 576) f32, mask (64, 1, 320, 576) f32, img_h, img_w
and returns the FULL output (64, 3, 3) f32.

Strategy (pure data parallel, 8 batches/core x 8 cores):
  The 1024 sample points form a fixed separable 32x32 grid.  The needed rows
  follow an affine pattern y0[k] = 16 + 37*(k//4) + 9*(k%4) (k=31 lands on
  rows 302/303 with wy patched to 1.0, exact since the true sample sits on
  row 303).  So the row fetch is 8 static 3D-access-pattern DMAs -- no
  gpsimd gather, no index tables.

  Per core:
    1. 4 flow DMAs (one per k%4 slot): [bc 16][k4 8][pair-row 1152] and 4
       mask DMAs [b 8][k4 8][1152] issued from sync/scalar HWDGE queues at
       t=0, plus one packed constants blob.
    2. x-lerp directly on strided column views (9 uniform-stride runs),
       then y-lerp -> sampF [128=(k4,b,c)][(s,i)=128], sampM [64=(k4,b)].
    3. One PE matmul fuses transpose + image scaling + grid offset + a
       compile-time Hartley normalization (constant T from the source grid;
       dst stats differ from it only by O(flow/image) ~ 1e-4 relative, and
       the solve's eps-regularization sensitivity to T is ~1e-8).
    4. D = [w, w*p, w*q, w*(p^2+q^2)] (raw normalized coords, no centering);
       moments = C9^T @ D in ONE PE matmul (stationary C9 [128,72] holds 9
       redundant features x 8 point-tiles); tile-diagonal extracted with 8
       copies + reduce.
    5. Transpose moments to [batch, moment]; assemble the 8x9 augmented
       normal equations directly with ~12 strided copies (no EQ matmul);
       unpivoted Gauss-Jordan; denormalize with immediate constants;
       sign/scale fix; support gate; DMA out (8,3,3).
"""

import dataclasses
import math
import numpy as np

import concourse.bass as bass
import concourse.bacc as bacc
import concourse.mybir as mybir
from concourse import tile
from concourse import bass_utils

F32 = mybir.dt.float32
ALU = mybir.AluOpType

NCORES = 8
BPC = 8          # batches per core
HF, WF = 320, 576
NG = 32          # grid is NG x NG points
NPTS = NG * NG
EPS = 1e-6

# constant-blob column layout
C_SXY = 0        # [128, 128] diag transpose-scale
C_C9 = 128       # [128, 72] point features (9F x 8 tiles)
C_WY4 = 200      # [128, 4] flow y-weights
C_G6 = 204       # [6, 128] grid-offset stationary
C_GR6 = 332      # [6, 128] grid-offset moving
C_IDN = 460      # [128, 128] identity
C_WX = 588       # [128, 32] x-weights
C_WY2M = 620     # [128, 2] mask y-weights
C_NCOL = 622


def _grid_1d(size, n):
    m = int(size * 0.05)
    return np.linspace(m, size - m - 1, n, dtype=np.float32)


def _wrap16(idxlist, nslots):
    """dma_gather index wrapping: list pos k -> partition k%16, slot k//16,
    replicated across the 8 gpsimd cores (16-partition groups)."""
    base = np.zeros((16, nslots), np.int16)
    for k, v in enumerate(idxlist):
        base[k % 16, k // 16] = v
    return np.tile(base, (8, 1))


class _Consts:
    def __init__(self, img_h, img_w):
        ys = _grid_1d(HF, NG)
        xs = _grid_1d(WF, NG)
        k = np.arange(NG)
        y0 = 16 + 37 * (k // 4) + 9 * (k % 4)
        x0 = np.floor(xs).astype(np.int64)
        wy = (ys.astype(np.float64) - y0)
        wy[31] = 1.0  # rows (302,303) loaded; true sample is row 303 exactly
        wx = (xs.astype(np.float64) - x0)
        # x0 structure: i=0..3 -> 28+[0,16,33,50]; i=4a'+r+4 -> 94+67a'+17r
        assert x0[0] == 28 and x0[1] == 44 and x0[2] == 61 and x0[3] == 78
        assert all(x0[4 + 4 * a + r] == 94 + 67 * a + 17 * r
                   for a in range(7) for r in range(4))
        self.x0 = x0
        sx = float(np.float32((img_w - 1) / max(WF - 1, 1)))
        sy = float(np.float32((img_h - 1) / max(HF - 1, 1)))

        # compile-time Hartley T from the source grid (used for src AND dst)
        gx = xs.astype(np.float64)[np.arange(NPTS) % NG]
        gy = ys.astype(np.float64)[np.arange(NPTS) // NG]
        sxi, syi = gx * sx, gy * sy
        mx, my = sxi.mean(), syi.mean()
        s_c = max(np.sqrt((sxi - mx) ** 2 + (syi - my) ** 2).mean()
                  / math.sqrt(2.0), 1e-8)
        a_t = 1.0 / s_c
        self.s_c, self.mx, self.my, self.a_t = s_c, mx, my, a_t

        u = (xs.astype(np.float64) * sx - mx) * a_t   # per i
        v = (ys.astype(np.float64) * sy - my) * a_t   # per k

        CB = np.zeros((128, C_NCOL), np.float64)
        # SXYDn: diag, n = (b*2+c)*8 + k4 -> scale = (sx|sy) * a_t
        n = np.arange(128)
        c_n = (n // 8) % 2
        k4_n = n % 8
        CB[:, C_SXY:C_SXY + 128] = np.eye(128) * np.where(c_n == 0, sx, sy) * a_t
        # C9[m=(s,i), t*9+f], F = (uu, uv, u, uv, vv, v, u, v, 1)
        m = np.arange(128)
        s_m, i_m = m // 32, m % 32
        for t in range(8):
            U = u[i_m]
            V = v[t * 4 + s_m]
            F9 = np.stack([U * U, U * V, U, U * V, V * V, V, U, V,
                           np.ones_like(U)], -1)  # (128, 9)
            CB[:, C_C9 + 9 * t:C_C9 + 9 * t + 9] = F9
        # WY4[p, s] = wy[(p%8)*4 + s]  (partition p = bc*8 + k4)
        CB[:, C_WY4:C_WY4 + 4] = wy[(np.arange(128) % 8)[:, None] * 4
                                    + np.arange(4)[None, :]]
        # G6/GR6: psF[m, n] += sum_r G6[r, m] * GR6[r, n]
        G6 = np.zeros((6, 128))
        GR6 = np.zeros((6, 128))
        for sp in range(4):
            G6[sp] = (s_m == sp)
            GR6[sp] = np.where(
                c_n == 1,
                (ys.astype(np.float64)[4 * k4_n + sp] * sy - my) * a_t, 0.0)
        G6[4] = xs.astype(np.float64)[i_m]
        GR6[4] = np.where(c_n == 0, sx * a_t, 0.0)
        G6[5] = 1.0
        GR6[5] = np.where(c_n == 0, -mx * a_t, 0.0)
        CB[0:6, C_G6:C_G6 + 128] = G6
        CB[0:6, C_GR6:C_GR6 + 128] = GR6
        CB[:, C_IDN:C_IDN + 128] = np.eye(128)
        CB[:, C_WX:C_WX + 32] = np.tile(wx[None, :], (128, 1))
        # mask: gather desc vv = t*128 + p -> s = 2t + p//64, (b,k4) = p%64
        p_ = np.arange(128)
        CB[:, C_WY2M:C_WY2M + 2] = wy[
            (p_ % 64 % 8)[:, None] * 4
            + 2 * np.arange(2)[None, :] + (p_ // 64)[:, None]]
        self.CB = CB.astype(np.float32)

        # gather index tables: desc vv = t*128 + p
        def row_f(p, s):
            bc, k4 = p // 8, p % 8
            return bc * HF + 16 + 37 * k4 + 9 * s

        giA = [row_f(vv % 128, vv // 128) for vv in range(256)]
        giB = [row_f(vv % 128, 2 + vv // 128) for vv in range(256)]

        def row_m(p, t):
            slo, bk = p // 64, p % 64
            b, k4 = bk // 8, bk % 8
            return b * HF + 16 + 37 * k4 + 9 * (2 * t + slo)

        giM = [row_m(vv % 128, vv // 128) for vv in range(256)]
        self.GT = np.concatenate(
            [_wrap16(giA, 16), _wrap16(giB, 16), _wrap16(giM, 16)],
            axis=1)  # [128, 48] i16


def _flat(ap):
    return ap.rearrange("b c h w -> (b c h w)").unsqueeze(0)


def _rows_view(ap, nrows, elem):
    """Overlapping rows view of a DRAM tensor: [(WF, nrows), (1, elem)]."""
    flat = _flat(ap)
    return dataclasses.replace(flat, ap=[[WF, nrows], [1, elem]])


def _build_program(cc: _Consts):
    nc = bacc.Bacc("TRN2", target_bir_lowering=False, debug=False,
                   num_swdge_queues=2)

    flow = nc.dram_tensor("flow", [BPC, 2, HF, WF], F32, kind="ExternalInput")
    mask = nc.dram_tensor("mask", [BPC, 1, HF, WF], F32, kind="ExternalInput")
    CBd = nc.dram_tensor("CB", [128, C_NCOL], F32, kind="ExternalInput")
    GTd = nc.dram_tensor("GT", [128, 48], mybir.dt.int16, kind="ExternalInput")
    Hout = nc.dram_tensor("H", [BPC, 3, 3], F32, kind="ExternalOutput")

    V = nc.vector
    A = nc.scalar
    T = nc.tensor
    S = nc.sync

    with tile.TileContext(nc) as tc:
        with (
            tc.tile_pool(name="sb", bufs=1) as pool,
            tc.tile_pool(name="ps", bufs=1, space="PSUM") as psp,
        ):
            # ---------------- DMAs in ----------------
            # gather index tables first (gate the SWDGE gathers), then the
            # constants blob; bulk rows via 3 gpsimd dma_gathers (descriptor
            # batches sustain ~2x the per-queue HWDGE descriptor-gen rate)
            GT_t = pool.tile([128, 48], mybir.dt.int16, tag="GT")
            A.dma_start(GT_t[:, :], GTd[:])
            CB_t = pool.tile([128, C_NCOL], F32, tag="CB")
            S.dma_start(CB_t[:, :], CBd[:])

            tF = pool.tile([128, 4, 1152], F32)   # [p=(b,c,k4)][s][pair row]
            tM = pool.tile([128, 2, 1152], F32)   # [p=(slo,b,k4)][t][pair row]
            G = nc.gpsimd
            G.dma_gather(
                out_ap=tF[:, 0:2, :],
                in_ap=_rows_view(flow.ap(), 2 * BPC * HF - 2, 1152),
                idxs_ap=GT_t[:, 0:16], num_idxs=256, num_idxs_reg=256,
                elem_size=1152, elem_step=WF, queue_num=0)
            G.dma_gather(
                out_ap=tF[:, 2:4, :],
                in_ap=_rows_view(flow.ap(), 2 * BPC * HF - 2, 1152),
                idxs_ap=GT_t[:, 16:32], num_idxs=256, num_idxs_reg=256,
                elem_size=1152, elem_step=WF, queue_num=1)
            G.dma_gather(
                out_ap=tM[:, :, :],
                in_ap=_rows_view(mask.ap(), BPC * HF - 1, 1152),
                idxs_ap=GT_t[:, 32:48], num_idxs=256, num_idxs_reg=256,
                elem_size=1152, elem_step=WF, queue_num=0)

            SXYDn = CB_t[:, C_SXY:C_SXY + 128]
            C9 = CB_t[:, C_C9:C_C9 + 72]
            WY4 = CB_t[:, C_WY4:C_WY4 + 4]
            G6 = CB_t[0:6, C_G6:C_G6 + 128]
            GR6 = CB_t[0:6, C_GR6:C_GR6 + 128]
            IDN = CB_t[:, C_IDN:C_IDN + 128]
            WXT = CB_t[:, C_WX:C_WX + 32]
            WY2M = CB_t[:, C_WY2M:C_WY2M + 2]

            IEYE = pool.tile([8, 9], F32, tag="IEYE")
            V.memset(IEYE[:, :], 0.0)
            V.memset(IEYE[:, 0:9:4], 1.0)

            # ---------------- interp (affine-x views) ----------------
            # x0 families: i=0,1 (base 28, step 16); i=2,3 (base 61, step 17);
            # i=4+4a'+r (base 94, strides 67/17).  x-lerp directly on strided
            # views of the raw pair rows (s,a merged), then y-lerp.
            def xy_interp(tile_t, np_, WXv, WYv, samp, XD, XL, s_lo, s_hi):
                ns = s_hi - s_lo
                nsa = 2 * ns
                flat = tile_t[:, :, :].rearrange("p s e -> p (s e)")
                xlf = XL[:, :, :].rearrange("p sa i -> p (sa i)")
                xdf = XD[:, :, :].rearrange("p sa i -> p (sa i)")
                for (fam_off, fam_i0, dims, odims) in (
                    (28, 0, [[16, 2]], [[1, 2]]),
                    (61, 2, [[17, 2]], [[1, 2]]),
                    (94, 4, [[67, 7], [17, 4]], [[4, 7], [1, 4]]),
                ):
                    src_ap = [list(flat.ap[0]), [576, nsa]] + \
                        [list(x) for x in dims]
                    g0 = dataclasses.replace(
                        flat, ap=[list(x) for x in src_ap],
                        offset=flat.offset + s_lo * 1152 + fam_off)
                    g1 = dataclasses.replace(
                        flat, ap=[list(x) for x in src_ap],
                        offset=flat.offset + s_lo * 1152 + fam_off + 1)
                    out_ap = [list(xlf.ap[0]), [32, nsa]] + \
                        [list(x) for x in odims]
                    d_o = dataclasses.replace(
                        xlf, ap=[list(x) for x in out_ap],
                        offset=xlf.offset + fam_i0)
                    d_t = dataclasses.replace(
                        xdf, ap=[list(x) for x in out_ap],
                        offset=xdf.offset + fam_i0)
                    if len(dims) == 2:
                        wxv = WXv[:, 4:32].rearrange("p (a r) -> p a r", a=7) \
                            .unsqueeze(1).broadcast_to([np_, nsa, 7, 4])
                    else:
                        wxv = WXv[:, fam_i0:fam_i0 + 2].unsqueeze(1) \
                            .broadcast_to([np_, nsa, 2])
                    V.tensor_tensor(out=d_t, in0=g1, in1=g0, op=ALU.subtract)
                    V.tensor_tensor(out=d_t, in0=d_t, in1=wxv, op=ALU.mult)
                    V.tensor_tensor(out=d_o, in0=d_t, in1=g0, op=ALU.add)
                # y-lerp
                XL4 = XL[:, :, :].rearrange("p (s a) i -> p s a i", a=2)
                wyv = WYv[:, s_lo:s_hi].unsqueeze(2).broadcast_to(
                    [np_, ns, 32])
                sv = samp[:, 32 * s_lo:32 * s_hi] \
                    .rearrange("p (s i) -> p s i", s=ns)
                dv = XD[:, 0:ns, :]
                V.tensor_tensor(out=dv, in0=XL4[:, :, 1, :],
                                in1=XL4[:, :, 0, :], op=ALU.subtract)
                V.tensor_tensor(out=dv, in0=dv, in1=wyv, op=ALU.mult)
                V.tensor_tensor(out=sv, in0=dv, in1=XL4[:, :, 0, :],
                                op=ALU.add)

            sampF = pool.tile([128, 128], F32)     # [(b,c,k4)][(s,i)]
            XD0 = pool.tile([128, 4, 32], F32)
            XL0 = pool.tile([128, 4, 32], F32)
            XD1 = pool.tile([128, 4, 32], F32)
            XL1 = pool.tile([128, 4, 32], F32)
            xy_interp(tF, 128, WXT, WY4, sampF, XD0, XL0, 0, 2)
            xy_interp(tF, 128, WXT, WY4, sampF, XD1, XL1, 2, 4)
            sampM = pool.tile([128, 64], F32)      # [(slo,b,k4)][(t,i)]
            XDM = pool.tile([128, 4, 32], F32)
            XLM = pool.tile([128, 4, 32], F32)
            xy_interp(tM, 128, WXT, WY2M, sampM, XDM, XLM, 0, 2)

            # ------- transpose + scale + grid + normalize (one PE pass) -----
            psF = psp.tile([128, 128], F32)
            T.matmul(psF[:, :], sampF[:, :], SXYDn, start=True, stop=False)
            T.matmul(psF[:, :], G6, GR6, start=False, stop=True)
            PQs = pool.tile([128, 128], F32)  # normalized dst [m=(s,i)][(b,c,t)]
            V.tensor_copy(PQs[:, :], psF[:, :])
            psM2 = psp.tile([64, 128], F32)
            T.transpose(psM2[:, :], sampM[:, :], IDN)
            # psM2[(t2,i)][(slo,b,k4)]; mask is nonnegative so w needs no
            # clamp -- write w straight into D's q0 block (4 quadrant copies
            # fold the point-partition remap and the (b,k4)->(t,b) permute)
            D = pool.tile([128, 256], F32)    # [m][(t,q,b)]
            Dv = D[:, :].rearrange("p (t q b) -> p q t b", q=4, b=8)
            for t2 in range(2):
                for slo in range(2):
                    dst = D[64 * t2 + 32 * slo:64 * t2 + 32 * slo + 32, :] \
                        .rearrange("p (t q b) -> p q t b", q=4, b=8)[:, 0, :, :]
                    src_ = psM2[32 * t2:32 * t2 + 32, :] \
                        .rearrange("p (s2 b t) -> p s2 t b",
                                   s2=2, b=8, t=8)[:, slo, :, :]
                    V.tensor_copy(dst, src_)
            d12 = D[:, :].rearrange("p (t q b) -> p t q b", q=4, b=8)[:, :, 1:3, :]
            pq12 = PQs[:, :].rearrange("p (b c t) -> p t c b", t=8, b=8, c=2)
            wb2 = Dv[:, 0, :, :].unsqueeze(2).broadcast_to([128, 8, 2, 8])
            V.tensor_tensor(out=d12, in0=pq12, in1=wb2, op=ALU.mult)
            SQ = pool.tile([128, 128], F32)
            V.tensor_tensor(out=SQ[:, :], in0=PQs[:, :], in1=PQs[:, :],
                            op=ALU.mult)
            R2 = pool.tile([128, 64], F32)
            sq3 = SQ[:, :].rearrange("p (b c t) -> p c t b", t=8, b=8, c=2)
            V.tensor_tensor(out=R2[:, :].rearrange("p (t b) -> p t b", t=8),
                            in0=sq3[:, 0, :, :], in1=sq3[:, 1, :, :], op=ALU.add)
            V.tensor_tensor(out=Dv[:, 3, :, :],
                            in0=R2[:, :].rearrange("p (t b) -> p t b", t=8),
                            in1=Dv[:, 0, :, :], op=ALU.mult)

            # ------- moments: psMom[f, (q,b)] = sum_t C9_t^T D_t ----------
            psMom = psp.tile([9, 32], F32)
            for t in range(8):
                T.matmul(psMom[:, :], C9[:, 9 * t:9 * t + 9],
                         D[:, 32 * t:32 * t + 32], start=(t == 0),
                         stop=(t == 7))
            Msb = pool.tile([9, 32], F32)
            V.tensor_copy(Msb[:, :], psMom[:, :])
            # per-q PE transposes: [8 = batch, 9 = feature] tiles
            MQT = [pool.tile([8, 9], F32, name=f"MQT{q}") for q in range(4)]
            psQ = [psp.tile([8, 9], F32, name=f"psQ{q}") for q in range(4)]
            for q in range(4):
                T.transpose(psQ[q][:, :], Msb[:, 8 * q:8 * q + 8],
                            IDN[0:9, 0:9])
                V.tensor_copy(MQT[q][:, :], psQ[q][:, :])
            M0, Mp, Mq, Mr = MQT

            # ---------------- assemble AUG [8, 72] ----------------
            AUG = pool.tile([8, 72], F32)
            V.memset(AUG[:, :], 0.0)
            a33 = AUG[:, :].rearrange("p (r c) -> p r c", r=8)

            def v33(mt, tr=False):
                vv = mt[:, :].rearrange("p (r c) -> p r c", r=3)
                return vv.rearrange("p r c -> p c r") if tr else vv

            V.tensor_copy(a33[:, 0:3, 0:3], v33(M0))
            V.tensor_copy(a33[:, 3:6, 3:6], v33(M0))
            V.tensor_scalar(out=a33[:, 0:3, 6:8], in0=v33(Mp)[:, :, 0:2],
                            scalar1=-1.0, op0=ALU.mult, scalar2=None)
            V.tensor_scalar(out=a33[:, 3:6, 6:8], in0=v33(Mq)[:, :, 0:2],
                            scalar1=-1.0, op0=ALU.mult, scalar2=None)
            V.tensor_scalar(out=a33[:, 6:8, 0:3], in0=v33(Mp, tr=True)[:, 0:2, :],
                            scalar1=-1.0, op0=ALU.mult, scalar2=None)
            V.tensor_scalar(out=a33[:, 6:8, 3:6], in0=v33(Mq, tr=True)[:, 0:2, :],
                            scalar1=-1.0, op0=ALU.mult, scalar2=None)
            V.tensor_copy(a33[:, 6:8, 6:8], v33(Mr)[:, 0:2, 0:2])
            V.tensor_copy(AUG[:, 8:27:9], Mp[:, 6:9])
            V.tensor_copy(AUG[:, 35:54:9], Mq[:, 6:9])
            V.tensor_scalar(out=AUG[:, 62:72:9], in0=Mr[:, 6:8],
                            scalar1=-1.0, op0=ALU.mult, scalar2=None)
            V.tensor_scalar(out=AUG[:, 0:71:10], in0=AUG[:, 0:71:10],
                            scalar1=EPS, op0=ALU.add, scalar2=None)
            # support gate from AUG[2,2] = sum(w) + eps
            GT = pool.tile([8, 1], F32)
            V.tensor_scalar(out=GT[:, :], in0=AUG[:, 20:21],
                            scalar1=NPTS * 1e-4 + EPS, op0=ALU.is_gt,
                            scalar2=None)

            # ------- Gauss-Jordan (unnormalized rows) -------
            FC = pool.tile([8, 8], F32)
            PIV = pool.tile([8, 1], F32)
            DRC = pool.tile([8, 8], F32)
            U8 = pool.tile([8, 72], F32)
            for kk in range(8):
                w_ = 9 - kk
                # F[i] = a_ik / a_kk, with F[k] forced to 0 so row k survives
                V.reciprocal(PIV[:, :], AUG[:, 9 * kk + kk:9 * kk + kk + 1])
                V.tensor_scalar(out=FC[:, :], in0=AUG[:, kk:72:9],
                                scalar1=PIV[:, :], op0=ALU.mult, scalar2=None)
                V.memset(FC[:, kk:kk + 1], 0.0)
                fcol = FC[:, :].unsqueeze(2).broadcast_to([8, 8, w_])
                rkb = AUG[:, 9 * kk + kk:9 * kk + 9].unsqueeze(1) \
                    .broadcast_to([8, 8, w_])
                ucols = U8[:, :].rearrange("p (r c) -> p r c", r=8)[:, :, 0:w_]
                acols = AUG[:, :].rearrange("p (r c) -> p r c", r=8)[:, :, kk:9]
                V.tensor_tensor(out=ucols, in0=fcol, in1=rkb, op=ALU.mult)
                V.tensor_tensor(out=acols, in0=acols, in1=ucols, op=ALU.subtract)

            # ---------------- denormalize (immediate T) ----------------
            c_ = V.tensor_copy
            HN = pool.tile([8, 9], F32)
            V.reciprocal(DRC[:, :], AUG[:, 0:72:10])
            V.tensor_tensor(out=HN[:, 0:8], in0=AUG[:, 8:72:9],
                            in1=DRC[:, :], op=ALU.mult)
            V.memset(HN[:, 8:9], 1.0)
            s_c, mx, my, a_t = cc.s_c, cc.mx, cc.my, cc.a_t
            T1 = pool.tile([8, 9], F32)
            H1 = pool.tile([8, 9], F32)
            V.tensor_scalar(out=T1[:, 0:3], in0=HN[:, 0:3], scalar1=s_c,
                            op0=ALU.mult, scalar2=None)
            V.scalar_tensor_tensor(out=H1[:, 0:3], in0=HN[:, 6:9], scalar=mx,
                                   in1=T1[:, 0:3], op0=ALU.mult, op1=ALU.add)
            V.tensor_scalar(out=T1[:, 3:6], in0=HN[:, 3:6], scalar1=s_c,
                            op0=ALU.mult, scalar2=None)
            V.scalar_tensor_tensor(out=H1[:, 3:6], in0=HN[:, 6:9], scalar=my,
                                   in1=T1[:, 3:6], op0=ALU.mult, op1=ALU.add)
            c_(H1[:, 6:9], HN[:, 6:9])
            H2 = pool.tile([8, 9], F32)
            H1v = H1[:, :].rearrange("p (r c) -> p r c", r=3)
            H2v = H2[:, :].rearrange("p (r c) -> p r c", r=3)
            V.tensor_scalar(out=H2v[:, :, 0:2], in0=H1v[:, :, 0:2],
                            scalar1=a_t, op0=ALU.mult, scalar2=None)
            T2 = pool.tile([8, 3], F32)
            T3 = pool.tile([8, 3], F32)
            V.tensor_scalar(out=T2[:, :], in0=H1[:, 0:9:3], scalar1=-mx * a_t,
                            op0=ALU.mult, scalar2=None)
            V.scalar_tensor_tensor(out=T3[:, :], in0=H1[:, 1:9:3],
                                   scalar=-my * a_t, in1=T2[:, :],
                                   op0=ALU.mult, op1=ALU.add)
            V.tensor_tensor(out=H2[:, 2:9:3], in0=T3[:, :], in1=H1[:, 2:9:3],
                            op=ALU.add)
            # sign/scale fix: H /= (H22 + sign(H22)*1e-8)
            ISN = pool.tile([8, 1], F32)
            DEN = pool.tile([8, 1], F32)
            RECD = pool.tile([8, 1], F32)
            V.tensor_scalar(out=ISN[:, :], in0=H2[:, 8:9], scalar1=0.0,
                            op0=ALU.is_lt, scalar2=-2e-8, op1=ALU.mult)
            V.tensor_scalar(out=ISN[:, :], in0=ISN[:, :], scalar1=1e-8,
                            op0=ALU.add, scalar2=None)
            V.tensor_tensor(out=DEN[:, :], in0=H2[:, 8:9], in1=ISN[:, :],
                            op=ALU.add)
            V.reciprocal(RECD[:, :], DEN[:, :])
            V.tensor_scalar(out=H2[:, :], in0=H2[:, :], scalar1=RECD[:, :],
                            op0=ALU.mult, scalar2=None)
            # support gate
            IG = pool.tile([8, 1], F32)
            TI = pool.tile([8, 9], F32)
            OUTt = pool.tile([8, 9], F32)
            V.tensor_scalar(out=IG[:, :], in0=GT[:, :], scalar1=-1.0,
                            op0=ALU.mult, scalar2=1.0, op1=ALU.add)
            V.tensor_scalar(out=TI[:, :], in0=IEYE[:, :], scalar1=IG[:, :],
                            op0=ALU.mult, scalar2=None)
            V.scalar_tensor_tensor(out=OUTt[:, :], in0=H2[:, :], scalar=GT[:, :],
                                   in1=TI[:, :], op0=ALU.mult, op1=ALU.add)
            S.dma_start(Hout.ap().rearrange("b r c -> b (r c)"), OUTt[:, :])

    nc.compile()
    return nc


# ---------------------------------------------------------------------------
# host wrapper
# ---------------------------------------------------------------------------

_CACHE = {}


def _get(img_h, img_w):
    key = (int(img_h), int(img_w))
    if key not in _CACHE:
        cc = _Consts(*key)
        _CACHE[key] = (cc, _build_program(cc))
    return _CACHE[key]


def _in_maps(cc, flow, mask):
    flow = np.ascontiguousarray(flow, np.float32)
    mask = np.ascontiguousarray(mask, np.float32)
    return [{
        "flow": flow[c * BPC:(c + 1) * BPC],
        "mask": mask[c * BPC:(c + 1) * BPC],
        "CB": cc.CB,
        "GT": cc.GT,
    } for c in range(NCORES)]


def run(flow, mask, img_h, img_w, trace=False, **spmd_kwargs):
    cc, nc = _get(img_h, img_w)
    res = bass_utils.run_bass_kernel_spmd(
        nc, _in_maps(cc, flow, mask), list(range(NCORES)), trace=trace,
        **spmd_kwargs)
    out = np.concatenate([res.results[c]["H"] for c in range(NCORES)], axis=0)
    return out.astype(np.float32), res


def kernel(flow, mask, img_h, img_w):
    out, _ = run(flow, mask, img_h, img_w)
    return out
